# revision 1
# baseline (speedup 1.0000x reference)
"""Trainium2 Bass kernel for nn_ProtoCycleModel (retrieval_knn).

Problem: P=65536 prototypes, C=64 classes, D=256.
Per class c (rows c::64 of each table, n=1024):
    p2_inv = (p2_c - b) @ inv(W.T)          # y-side of direction "source"
    p1_fwd = p1_c @ W.T + b                 # y-side of direction "target"
    loss_src[c] = mean_i min_j ||p1_c[i] - p2_inv[j]||^2
    loss_tgt[c] = mean_i min_j ||p2_c[i] - p1_fwd[j]||^2
Output: (2, 64) fp32.

Sharding: class axis across 8 cores (8 classes/core). Each core:
  - loads its (8*1024, 256) slices of both tables (row-major, contiguous)
  - PE-transposes them to d-major (fp32 exact)
  - computes transformed tables directly in transposed space:
        yT = Mat @ xT + bias   (Mat = -2*inv(W.T)-style, folded scale -2)
    so the pairwise matmul G = xT.T @ yT gives -2 * x.y' directly.
  - |y'|^2 row: ones-matmul over Square(transform psum + bias) (scale 1/4
    baked into a 0.25-constant stationary matrix), broadcast to all 128
    partitions for free via M=128 stationary ones.
  - per i-tile: fused DVE tensor_tensor_reduce: min_j (G + |y'|^2) -> [128,1]
  - per-class scalars via ones-matmul cross-partition sum; host gathers.

All matmuls run in float32r (TF32-like, full PE rate at N>=512, ~16x more
accurate than bf16). Everything else fp32.
"""

import numpy as np

P, C, D = 65536, 64, 256
N_CORES = 8
CPC = C // N_CORES          # classes per core = 8
NPC = P // C                # prototypes per class = 1024
IT = NPC // 128             # i-tiles per class = 8

# ys application mode: "ttr" = fused DVE tensor_tensor_reduce;
# "fold" = K=1 matmul folds ys row into PSUM, then plain tensor_reduce.
YS_MODE = "fold"
import os as _os
PSG_WIDE = _os.environ.get("K_PSG_WIDE", "0") == "1"   # [128,1024] G tiles
PSG_BUFS = int(_os.environ.get("K_PSG_BUFS", "4"))
PSM_BUFS = int(_os.environ.get("K_PSM_BUFS", "2"))

_CACHE = {}


def _build_bass():
    import concourse.bass as bass
    from concourse import bacc
    import concourse.tile as tile
    from concourse import mybir
    from concourse.masks import make_identity

    FP32 = mybir.dt.float32
    FP32R = mybir.dt.float32r
    BF16 = mybir.dt.bfloat16
    AF = mybir.ActivationFunctionType
    ALU = mybir.AluOpType
    AX = mybir.AxisListType

    nc = bacc.Bacc(None, target_bir_lowering=False)

    p1_d = nc.dram_tensor("p1", [CPC * NPC, D], FP32, kind="ExternalInput")
    p2_d = nc.dram_tensor("p2", [CPC * NPC, D], FP32, kind="ExternalInput")
    # mats[dir][kchunk] : [128, 256] fp32, lhsT layout [d, d'] with the -2
    # scale folded in.  dir 0 = source (V2 = -2*inv(W.T)), dir 1 = target
    # (Wt2 = -2*W.T).
    mats_d = nc.dram_tensor("mats", [2, 2, 128, D], FP32, kind="ExternalInput")
    consts_d = nc.dram_tensor("consts", [128, 385], FP32, kind="ExternalInput")
    # biases[dir] : [128, 2] fp32 (column = d' chunk);  dir0 = +2*(b@V),
    # dir1 = -2*b.
    bias_d = nc.dram_tensor("biases", [2, 128, 2], FP32, kind="ExternalInput")
    out_d = nc.dram_tensor("out", [1, 2 * CPC], FP32, kind="ExternalOutput")

    with tile.TileContext(nc) as tc:
        with (
            tc.tile_pool(name="const", bufs=1) as const,
            tc.tile_pool(name="xrow", bufs=6) as xrow_p,
            tc.tile_pool(name="xt", bufs=10) as xt_p,
            tc.tile_pool(name="yt", bufs=8) as yt_p,
            tc.tile_pool(name="sq", bufs=4) as sq_p,
            tc.tile_pool(name="ysb", bufs=4) as ysb_p,
            tc.tile_pool(name="scr", bufs=3) as scr_p,
            tc.tile_pool(name="psg", bufs=PSG_BUFS, space="PSUM") as psg_p,
            tc.tile_pool(name="psm", bufs=PSM_BUFS, space="PSUM") as psm_p,
        ):
            # ---- constants ----
            cblk_raw = const.tile([128, 385], FP32)
            nc.scalar.dma_start(cblk_raw[:], consts_d[:])
            cblk = const.tile([128, 385], FP32R)
            nc.vector.tensor_copy(cblk[:], cblk_raw[:])

            mats_raw = const.tile([128, 2, 2, D], FP32)
            nc.scalar.dma_start(mats_raw[:], mats_d[:].rearrange("a b p d -> p a b d"))
            mats = const.tile([128, 2, 2, D], FP32R)
            nc.vector.tensor_copy(mats[:], mats_raw[:])

            biases = const.tile([128, 2, 2], FP32)  # [p, dir, dchunk]
            nc.scalar.dma_start(biases[:], bias_d[:].rearrange("a p c -> p a c"))
            identr = cblk[:, 0:128]
            identf = cblk_raw[:, 0:128]
            ones1r = cblk[:, 128:129]
            ones_q = cblk[:, 257:385]

            pmin = const.tile([128, 2 * CPC * IT], FP32)   # col = dir*64+c*8+it
            pmin2 = (const.tile([128, 2 * CPC * IT], FP32, name="pmin2")
                     if not PSG_WIDE else pmin)
            pxs = const.tile([128, 2 * CPC * 2], FP32)     # col = dir*16+c*2+dc

            onesrow = cblk[0:1, 128:256]

            # ---- main loop: software-pipelined (prep one class ahead) ----
            state = {}

            def prep(c):
                xts = [[None, None], [None, None]]  # [table][dchunk]
                for t in range(2):
                    src_d = p1_d if t == 0 else p2_d
                    xr = xrow_p.tile([128, IT, D], FP32, tag="xrow", bufs=3)
                    xrr = xrow_p.tile([128, IT, D], FP32R, tag="xrowr", bufs=4)
                    half = NPC // 2
                    for hh in range(2):
                        nc.sync.dma_start(
                            xr[:, hh * (IT // 2):(hh + 1) * (IT // 2), :],
                            src_d[c * NPC + hh * half:
                                  c * NPC + (hh + 1) * half, :].rearrange(
                                "(k p) d -> p k d", p=128),
                        )
                        nc.vector.tensor_copy(
                            xrr[:, hh * (IT // 2):(hh + 1) * (IT // 2), :],
                            xr[:, hh * (IT // 2):(hh + 1) * (IT // 2), :])
                    for dc in range(2):
                        pst = psm_p.tile([128, 1024], FP32R, tag="misc")
                        for k in range(IT):
                            nc.tensor.transpose(
                                pst[:, k * 128:(k + 1) * 128],
                                xrr[:, k, dc * 128:(dc + 1) * 128],
                                identr,
                            )
                        xt_t = xt_p.tile([128, NPC], FP32R, tag="xt")
                        nc.scalar.copy(xt_t[:], pst[:])
                        xts[t][dc] = xt_t
                        # xs partials: sum_i x^2 per d-partition
                        trash = scr_p.tile([128, NPC], BF16, tag="scr")
                        nc.scalar.activation(
                            trash[:], xt_t[:], AF.Square,
                            accum_out=pxs[:, t * 16 + c * 2 + dc:
                                          t * 16 + c * 2 + dc + 1],
                        )

                yts_all = [[], []]
                ysrow_all = [None, None]
                for dr in range(2):
                    ysrc = xts[1 - dr]    # dir0: y from p2; dir1: y from p1
                    sqs = []
                    for dcp in range(2):   # output d' chunk
                        pstf = psm_p.tile([128, 1024], FP32, tag="misc")
                        for dc in range(2):
                            for ih in range(2):
                                nc.tensor.matmul(
                                    pstf[:, ih * 512:(ih + 1) * 512],
                                    mats[:, dr, dc, dcp * 128:(dcp + 1) * 128],
                                    ysrc[dc][:, ih * 512:(ih + 1) * 512],
                                    start=(dc == 0), stop=(dc == 1),
                                )
                        bias_ap = biases[:, dr, dcp:dcp + 1]
                        yt_t = yt_p.tile([128, NPC], FP32R, tag="yt")
                        nc.scalar.activation(
                            yt_t[:], pstf[:], AF.Identity, bias=bias_ap, scale=1.0)
                        sq_t = sq_p.tile([128, NPC], FP32R, tag="sq")
                        nc.scalar.activation(
                            sq_t[:], pstf[:], AF.Square, bias=bias_ap, scale=1.0)
                        yts_all[dr].append(yt_t)
                        sqs.append(sq_t)

                    psy = psm_p.tile([128, 1024], FP32, tag="misc")
                    for jh in range(2):
                        for dcp in range(2):
                            nc.tensor.matmul(
                                psy[0:1, jh * 512:(jh + 1) * 512],
                                ones_q[:, 0:1],
                                sqs[dcp][:, jh * 512:(jh + 1) * 512],
                                start=(dcp == 0), stop=(dcp == 1),
                            )
                    ysrow = ysb_p.tile([1, NPC], FP32R, tag="ysrow")
                    nc.scalar.copy(ysrow[:], psy[0:1, :])
                    ysrow_all[dr] = ysrow
                state[c] = (xts, yts_all, ysrow_all)

            def pairwise(c):
                xts, yts_all, ysrow_all = state.pop(c)
                for dr in range(2):
                    xside = xts[dr]       # dir0: x = p1; dir1: x = p2
                    yts = yts_all[dr]
                    ysrow = ysrow_all[dr]
                    for it in range(IT):
                        col = dr * 64 + c * 8 + it
                        pgs = [psg_p.tile([128, 512], FP32, tag="g",
                                          name=f"pg{jh}")
                               for jh in range(2)]
                        for dc in range(2):          # stationary reused 2x
                            for jh in range(2):
                                nc.tensor.matmul(
                                    pgs[jh][:],
                                    xside[dc][:, it * 128:(it + 1) * 128],
                                    yts[dc][:, jh * 512:(jh + 1) * 512],
                                    start=(dc == 0), stop=False,
                                )
                        for jh in range(2):          # ys fold, ones stationary
                            nc.tensor.matmul(
                                pgs[jh][:],
                                onesrow,
                                ysrow[:, jh * 512:(jh + 1) * 512],
                                start=False, stop=True,
                            )
                        for jh in range(2):
                            dst = pmin if jh == 0 else pmin2
                            nc.vector.tensor_reduce(
                                out=dst[:, col:col + 1], in_=pgs[jh][:],
                                axis=AX.X, op=ALU.min,
                            )

            prep(0)
            for c in range(CPC):
                if c + 1 < CPC:
                    prep(c + 1)
                pairwise(c)

            # ---- finals ----
            if PSG_WIDE:
                pminc = pmin
            else:
                pminc = const.tile([128, 2 * CPC * IT], FP32, name="pminc")
                nc.vector.tensor_tensor(
                    out=pminc[:], in0=pmin[:], in1=pmin2[:], op=ALU.min)
            red_min = const.tile([128, 16], FP32)
            nc.vector.tensor_reduce(
                out=red_min[:], in_=pminc[:].rearrange("p (g k) -> p g k", k=IT),
                axis=AX.X, op=ALU.add)
            red_xs = const.tile([128, 16], FP32)
            nc.vector.tensor_reduce(
                out=red_xs[:], in_=pxs[:].rearrange("p (g k) -> p g k", k=2),
                axis=AX.X, op=ALU.add)
            red = const.tile([128, 16], FP32R)
            nc.vector.tensor_tensor(
                out=red[:], in0=red_min[:], in1=red_xs[:], op=ALU.add)
            psf = psm_p.tile([1, 16], FP32, tag="misc")
            nc.tensor.matmul(psf[:], ones1r, red[:], start=True, stop=True)
            outrow = const.tile([1, 16], FP32)
            nc.scalar.mul(outrow[:], psf[:], 1.0 / NPC)
            nc.sync.dma_start(out_d[:], outrow[:])

    nc.compile()
    return nc


def _get_nc():
    if "nc" not in _CACHE:
        _CACHE["nc"] = _build_bass()
    return _CACHE["nc"]


def kernel(protos1, protos2, W, b, num_classes):
    from concourse.bass_utils import run_bass_kernel_spmd

    nc_classes = int(num_classes)
    assert nc_classes == C and protos1.shape == (P, D)

    protos1 = np.ascontiguousarray(protos1, dtype=np.float32)
    protos2 = np.ascontiguousarray(protos2, dtype=np.float32)
    W = np.asarray(W, dtype=np.float32)
    b = np.asarray(b, dtype=np.float32)

    # host-side tiny prep: inverse + scaled transform matrices
    V = np.linalg.inv(W.T.astype(np.float64)).astype(np.float32)  # (p2-b)@V
    V2 = (-2.0 * V).astype(np.float32)                 # lhsT [d, d'] dir0
    Wt2 = (-2.0 * W.T).astype(np.float32)              # lhsT [d, d'] dir1
    bias0 = (2.0 * (b.astype(np.float64) @ V.astype(np.float64))).astype(np.float32)
    bias1 = (-2.0 * b).astype(np.float32)
    mats = np.stack([
        np.stack([V2[0:128, :], V2[128:256, :]]),
        np.stack([Wt2[0:128, :], Wt2[128:256, :]]),
    ]).astype(np.float32)                               # [2, 2, 128, 256]
    idb = np.eye(128, dtype=np.float32)
    consts = np.concatenate([
        idb,
        np.ones((128, 129), dtype=np.float32),
        np.full((128, 128), 0.25, dtype=np.float32),
    ], axis=1)
    biases = np.stack([
        bias0.reshape(2, 128).T,                        # [128, 2] cols = chunk
        bias1.reshape(2, 128).T,
    ]).astype(np.float32)                               # [2, 128, 2]

    # class-major reordering: (P, D) -> (C, NPC, D)
    p1c = np.ascontiguousarray(protos1.reshape(NPC, C, D).transpose(1, 0, 2))
    p2c = np.ascontiguousarray(protos2.reshape(NPC, C, D).transpose(1, 0, 2))

    in_maps = []
    for core in range(N_CORES):
        sl = slice(core * CPC, (core + 1) * CPC)
        in_maps.append({
            "p1": np.ascontiguousarray(p1c[sl].reshape(CPC * NPC, D)),
            "p2": np.ascontiguousarray(p2c[sl].reshape(CPC * NPC, D)),
            "mats": mats,
            "biases": biases,
            "consts": consts,
        })

    nc = _get_nc()
    res = run_bass_kernel_spmd(nc, in_maps, core_ids=list(range(N_CORES)))
    _CACHE["last_result"] = res

    out = np.zeros((2, C), dtype=np.float32)
    for core in range(N_CORES):
        row = res.results[core]["out"].reshape(2, CPC)
        out[:, core * CPC:(core + 1) * CPC] = row
    return out



# revision 11
# speedup vs baseline: 1.2116x; 1.2116x over previous
"""Trainium2 Bass kernel for nn_ProtoCycleModel (retrieval_knn).

Problem: P=65536 prototypes, C=64 classes, D=256.
Per class c (rows c::64 of each table, n=1024):
    p2_inv = (p2_c - b) @ inv(W.T)          # y-side of direction "source"
    p1_fwd = p1_c @ W.T + b                 # y-side of direction "target"
    loss_src[c] = mean_i min_j ||p1_c[i] - p2_inv[j]||^2
    loss_tgt[c] = mean_i min_j ||p2_c[i] - p1_fwd[j]||^2
Output: (2, 64) fp32.

Sharding: class axis across 8 cores (8 classes/core).

v2 design (per core, per class, both directions dr in {0,1}):
  - Host passes class-major bf16 copies of both tables; the device loads
    them with XBAR DMA-transpose directly into d-major SBUF tiles
    xtb[t] = [128 d_lo, 2 d_chunk, 1024 i] bf16 (no PE transposes).
  - x~ = fp8(xtb)  (Act copy)  -> DoubleRow G stationary.
  - Transform (PE, fp32r stationary mats = -2*s_y[dr]*T_dr, moving bf16):
      pstf[dr,dcp] = -2*s_y*(linear part of y), bias added on the way out.
  - yt[dr] = fp8(pstf + bias)   [Act]  -> DoubleRow G moving.
  - sq = Square(pstf*sqrt_c + bias*sqrt_c) [Act], ysrow ones-matmul (PE,
    M=128 -> broadcast), ys_sb = copy to SBUF [DVE].  ys = s_y*|y_j|^2
    computed from the UNquantized transform (critical for accuracy).
  - G tile per (dr, i-tile): one DoubleRow fp8 matmul per 512-bank
    (K=256 in one instruction, 0.5 cycles/row).
  - min_j(G + ys): even i-tiles via DVE tensor_tensor_reduce (ys as the
    second operand), odd i-tiles via PE K=1 ys-fold matmul + gpsimd
    tensor_reduce(min) - balances DVE/Pool.
  - Scalar |x_i|^2 term and all unscaling are applied on the host
    (loss = psf/(1024*s_y) + mean_i|x_i|^2).
"""

import numpy as np

P, C, D = 65536, 64, 256
N_CORES = 8
CPC = C // N_CORES          # classes per core = 8
NPC = P // C                # prototypes per class = 1024
IT = NPC // 128             # i-tiles per class = 8

_CACHE = {}


def _build_bass():
    import concourse.bass as bass
    from concourse import bacc
    import concourse.tile as tile
    from concourse import mybir

    FP32 = mybir.dt.float32
    FP32R = mybir.dt.float32r
    BF16 = mybir.dt.bfloat16
    FP8 = mybir.dt.float8e4
    AF = mybir.ActivationFunctionType
    ALU = mybir.AluOpType
    AX = mybir.AxisListType
    DR_MODE = mybir.MatmulPerfMode.DoubleRow

    nc = bacc.Bacc(None, target_bir_lowering=False)

    p1b_d = nc.dram_tensor("p1b", [CPC * NPC, D], BF16, kind="ExternalInput")
    p2b_d = nc.dram_tensor("p2b", [CPC * NPC, D], BF16, kind="ExternalInput")
    # mats[dr][dc][dcp] : [128,128] bf16 lhsT chunk of -2*s_y[dr]*T_dr
    mats_d = nc.dram_tensor("mats", [2, 2, 2, 128, 128], BF16,
                            kind="ExternalInput")
    # constsr: [:,0:128] ones (ysrow lhsT / fold row), [:,128:129] ones col
    constsr_d = nc.dram_tensor("constsr", [128, 129], FP32R,
                               kind="ExternalInput")
    # constsf cols: 0-3 bias_dev[dr][dcp]; 4-7 bias_sq[dr][dcp]; 8-9 sqrt_c[dr]
    constsf_d = nc.dram_tensor("constsf", [128, 10], FP32, kind="ExternalInput")
    out_d = nc.dram_tensor("out", [1, 2 * CPC], FP32, kind="ExternalOutput")

    with tile.TileContext(nc) as tc:
        with (
            tc.tile_pool(name="const", bufs=1) as const,
            tc.tile_pool(name="xb", bufs=4) as xb_p,
            tc.tile_pool(name="xq", bufs=4) as xq_p,
            tc.tile_pool(name="yt", bufs=4) as yt_p,
            tc.tile_pool(name="sq", bufs=4) as sq_p,
            tc.tile_pool(name="ys", bufs=4) as ys_p,
            tc.tile_pool(name="gc", bufs=3) as gc_p,
            tc.tile_pool(name="psg", bufs=2, space="PSUM") as psg_p,
            tc.tile_pool(name="psm", bufs=2, space="PSUM") as psm_p,
        ):
            # ---- constants ----
            cr = const.tile([128, 129], FP32R)
            nc.scalar.dma_start(cr[:], constsr_d[:])
            cf = const.tile([128, 10], FP32)
            nc.scalar.dma_start(cf[:], constsf_d[:])
            mats = const.tile([128, 2, 2, 2, 128], BF16)
            nc.scalar.dma_start(
                mats[:], mats_d[:].rearrange("a b c p d -> p a b c d"))
            ones128 = cr[:, 0:128]
            ones_row = cr[0:1, 0:128]
            ones_col = cr[:, 128:129]

            # per-unit min columns: col = (ci*2 + dr)*8 + it.  All accums
            # come from DVE tensor_scalar ops (in-order engine).
            pmin = const.tile([128, 128], FP32, name="pmin")
            dumf = const.tile([128, 1], FP32, name="dumf")
            dumb = const.tile([128, 1], BF16, name="dumb")

            state = {}

            def emit_dmas(c):
                """DMA-transpose class-c rows of both tables into d-major."""
                xtbs = []
                for t, src in ((0, p1b_d), (1, p2b_d)):
                    xtb = xb_p.tile([128, 2, NPC], BF16, tag=f"xb{t}", name=f"xtb{t}")
                    for h in range(2):
                        nc.sync.dma_start_transpose(
                            xtb[:, :, h * 512:(h + 1) * 512],
                            src[c * NPC + h * 512:c * NPC + (h + 1) * 512, :],
                        )
                    xtbs.append(xtb)
                state[("xtb", c)] = xtbs

            def prep_ops(c):
                """Generator of (engine_tag, thunk) prep ops for class c."""
                xtbs = state[("xtb", c)]
                xqs = [None, None]
                yts = [None, None]
                yss = [None, None]
                sqs = [[None, None], [None, None]]
                psys = [None, None]
                pstfs = [[None, None], [None, None]]
                state[("res", c)] = (xqs, yts, yss)

                def quant(t):
                    xq = xq_p.tile([128, 2, NPC], FP8, tag=f"xq{t}", name=f"xq{t}")
                    nc.gpsimd.dma_start(xq[:], xtbs[t][:])
                    xqs[t] = xq

                def tf(dr, dcp, dc, jh):
                    if dc == 0 and jh == 0:
                        pstfs[dr][dcp] = psm_p.tile(
                            [128, NPC], FP32, tag="psm", name="pstf")
                    src = xtbs[1 - dr]  # dir0 transforms p2, dir1 p1
                    nc.tensor.matmul(
                        pstfs[dr][dcp][:, jh * 512:(jh + 1) * 512],
                        mats[:, dr, dc, dcp, :],
                        src[:, dc, jh * 512:(jh + 1) * 512],
                        start=(dc == 0), stop=(dc == 1),
                    )

                def ytf(dr, dcp):
                    if dcp == 0:
                        yts[dr] = yt_p.tile([128, 2, NPC], FP8, tag=f"yt{dr}", name=f"yt{dr}")
                    nc.scalar.activation(
                        yts[dr][:, dcp, :], pstfs[dr][dcp][:], AF.Identity,
                        bias=cf[:, dr * 2 + dcp:dr * 2 + dcp + 1], scale=1.0)

                def sqf(dr, dcp):
                    sq_t = sq_p.tile([128, NPC], FP32R, tag="sq", name="sq_t")
                    nc.scalar.activation(
                        sq_t[:], pstfs[dr][dcp][:], AF.Square,
                        bias=cf[:, 4 + dr * 2 + dcp:4 + dr * 2 + dcp + 1],
                        scale=cf[:, 8 + dr:9 + dr])
                    sqs[dr][dcp] = sq_t

                def ysrow(dr, jh, dcp):
                    if jh == 0 and dcp == 0:
                        psys[dr] = psm_p.tile([128, NPC], FP32, tag="psm", name="psy")
                    nc.tensor.matmul(
                        psys[dr][:, jh * 512:(jh + 1) * 512],
                        ones128,
                        sqs[dr][dcp][:, jh * 512:(jh + 1) * 512],
                        start=(dcp == 0), stop=(dcp == 1),
                    )

                def ysb(dr):
                    ys_t = ys_p.tile([128, NPC], FP32R, tag=f"ys{dr}", name=f"ys{dr}")
                    nc.scalar.copy(ys_t[:], psys[dr][:])
                    yss[dr] = ys_t

                # (engine, thunk) in dependency order; engine tags are only
                # informational for interleaving.
                ops = []
                ops.append(("act", lambda: quant(0)))
                ops.append(("act", lambda: quant(1)))
                for dr in range(2):
                    for dcp in range(2):
                        for jh in range(2):
                            for dc in range(2):
                                ops.append(("pe", (lambda a, b_, c_, d_:
                                            lambda: tf(a, b_, c_, d_))(
                                                dr, dcp, dc, jh)))
                        ops.append(("act", (lambda a, b_: lambda: ytf(a, b_))(
                            dr, dcp)))
                        ops.append(("act", (lambda a, b_: lambda: sqf(a, b_))(
                            dr, dcp)))
                    for jh in range(2):
                        for dcp in range(2):
                            ops.append(("pe", (lambda a, b_, c_:
                                        lambda: ysrow(a, b_, c_))(
                                            dr, jh, dcp)))
                    ops.append(("dve", (lambda a: lambda: ysb(a))(dr)))
                return ops

            def pairwise_units(c):
                """Generator of per-unit thunks for class c."""
                xqs, yts, yss = state.pop(("res", c))
                state.pop(("xtb", c))
                ci = c

                def unit(dr, it):
                    xside = xqs[dr]     # dir0 x = p1, dir1 x = p2
                    pg = psg_p.tile([128, NPC], FP32, tag="g", name="pg")
                    for jh in range(2):
                        # DoubleRow fp8 G (K=256 in one op), then fold ys
                        # into the same psum bank via a K=1 ones matmul.
                        nc.tensor.matmul(
                            pg[:, jh * 512:(jh + 1) * 512],
                            xside[:, :, it * 128:(it + 1) * 128],
                            yts[dr][:, :, jh * 512:(jh + 1) * 512],
                            start=True, stop=False,
                            perf_mode=DR_MODE,
                        )
                        nc.tensor.matmul(
                            pg[:, jh * 512:(jh + 1) * 512],
                            ones_row,
                            yss[dr][0:1, jh * 512:(jh + 1) * 512],
                            start=False, stop=True,
                        )
                    col = (ci * 2 + dr) * 8 + it
                    if it % 4 != 3:
                        # direct: DVE min-reduce straight from PSUM
                        nc.vector.tensor_scalar(
                            out=dumf.broadcast_to((128, NPC)),
                            in0=pg[:], scalar1=0.0, scalar2=None,
                            op0=ALU.add, op1=ALU.min,
                            accum_out=pmin[:, col:col + 1])
                    else:
                        # offloaded: Act copies psum->bf16 SBUF, DVE mins
                        # at 4x (2-byte packed, all-SBUF)
                        gc = gc_p.tile([128, NPC], BF16, tag="gc", name="gc")
                        nc.scalar.copy(gc[:], pg[:])
                        nc.vector.tensor_scalar(
                            out=dumb.broadcast_to((128, NPC)),
                            in0=gc[:], scalar1=0.0, scalar2=None,
                            op0=ALU.add, op1=ALU.min,
                            accum_out=pmin[:, col:col + 1])

                return [(dr, it, (lambda a, b_: lambda: unit(a, b_))(dr, it))
                        for dr in (0, 1) for it in range(8)]

            # ---- software-pipelined main loop ----
            # D-path units write pminD cols with running-min per slot?  No:
            # each (dr, it) has its own (col, path) slot: it//2 in 0..3,
            # even it -> D, odd it -> P.  Each col written exactly twice?
            # it=0,2 -> slots 0,1 (D); it=4,6 -> slots 2,3 (D);
            # it=1,3 -> slots 0,1 (P); it=5,7 -> slots 2,3 (P).  Unique. OK.
            emit_dmas(0)
            prep_queue = prep_ops(0)
            for op in prep_queue:
                op[1]()
            for c in range(CPC):
                units = pairwise_units(c)
                if c + 1 < CPC:
                    emit_dmas(c + 1)
                    prep_queue = prep_ops(c + 1)
                else:
                    prep_queue = []
                # interleave: after each unit, emit a slice of prep ops
                nu = len(units)
                np_ops = len(prep_queue)
                done = 0
                for ui, (dr, it, thunk) in enumerate(units):
                    thunk()
                    want = (ui + 1) * np_ops // nu
                    while done < want:
                        prep_queue[done][1]()
                        done += 1

            # ---- finals ----
            # sum the 8 i-tiles per (class, dir), then cross-partition sum.
            red = const.tile([128, 16], FP32R, name="red")
            with nc.allow_low_precision(reason="fp32r is bit-identical fp32"):
                nc.vector.tensor_reduce(
                    out=red[:],
                    in_=pmin[:].rearrange("p (g k) -> p g k", k=8),
                    axis=AX.X, op=ALU.add)
            psf = psm_p.tile([1, 16], FP32, tag="psm", name="psf")
            nc.tensor.matmul(psf[:], ones_col, red[:], start=True, stop=True)
            outrow = const.tile([1, 16], FP32)
            nc.scalar.copy(outrow[:], psf[:])
            nc.sync.dma_start(out_d[:], outrow[:])

    nc.compile()
    return nc


def _get_nc():
    if "nc" not in _CACHE:
        _CACHE["nc"] = _build_bass()
    return _CACHE["nc"]


def kernel(protos1, protos2, W, b, num_classes):
    import ml_dtypes
    from concourse.bass_utils import run_bass_kernel_spmd

    nc_classes = int(num_classes)
    assert nc_classes == C and protos1.shape == (P, D)

    protos1 = np.ascontiguousarray(protos1, dtype=np.float32)
    protos2 = np.ascontiguousarray(protos2, dtype=np.float32)
    W = np.asarray(W, dtype=np.float32)
    b = np.asarray(b, dtype=np.float32)

    # host-side prep: inverse, scales, transform matrices
    V = np.linalg.inv(W.T.astype(np.float64)).astype(np.float32)  # (p2-b)@V
    B0 = (np.linalg.norm(protos2 - b, axis=1).max()
          * np.linalg.norm(V, axis=0).max())
    B1 = (np.linalg.norm(protos1, axis=1).max()
          * np.linalg.norm(W, axis=1).max() + np.abs(b).max())
    s_y = np.array([56.0 / B0, 56.0 / B1], np.float64)

    import ml_dtypes as _mld
    mats = np.zeros((2, 2, 2, 128, 128), _mld.bfloat16)
    for dr, T in ((0, V), (1, W.T.copy())):
        M = (-2.0 * s_y[dr]) * T.astype(np.float64)
        for dc in range(2):
            for dcp in range(2):
                mats[dr, dc, dcp] = M[dc * 128:(dc + 1) * 128,
                                      dcp * 128:(dcp + 1) * 128]

    bias_dev = np.zeros((2, 256), np.float64)
    bias_dev[0] = 2.0 * s_y[0] * (b.astype(np.float64) @ V.astype(np.float64))
    bias_dev[1] = -2.0 * s_y[1] * b
    sqrt_c = np.sqrt(1.0 / (4.0 * s_y))          # per dir

    constsf = np.zeros((128, 10), np.float32)
    for dr in range(2):
        for dcp in range(2):
            col = bias_dev[dr, dcp * 128:(dcp + 1) * 128]
            constsf[:, dr * 2 + dcp] = col
            constsf[:, 4 + dr * 2 + dcp] = col * sqrt_c[dr]
        constsf[:, 8 + dr] = sqrt_c[dr]
    constsr = np.concatenate(
        [np.ones((128, 128), np.float32), np.ones((128, 1), np.float32)],
        axis=1)

    # class-major reordering: (P, D) -> (C, NPC, D), bf16 copies
    p1c = np.ascontiguousarray(protos1.reshape(NPC, C, D).transpose(1, 0, 2))
    p2c = np.ascontiguousarray(protos2.reshape(NPC, C, D).transpose(1, 0, 2))
    p1bf = p1c.astype(ml_dtypes.bfloat16)
    p2bf = p2c.astype(ml_dtypes.bfloat16)

    # host-side |x|^2 means per (dir, class)
    xs0 = (p1c.astype(np.float64) ** 2).sum(axis=2).mean(axis=1)  # (C,)
    xs1 = (p2c.astype(np.float64) ** 2).sum(axis=2).mean(axis=1)

    in_maps = []
    for core in range(N_CORES):
        sl = slice(core * CPC, (core + 1) * CPC)
        in_maps.append({
            "p1b": np.ascontiguousarray(p1bf[sl].reshape(CPC * NPC, D)),
            "p2b": np.ascontiguousarray(p2bf[sl].reshape(CPC * NPC, D)),
            "mats": mats,
            "constsr": constsr,
            "constsf": constsf,
        })

    nc = _get_nc()
    res = run_bass_kernel_spmd(nc, in_maps, core_ids=list(range(N_CORES)))
    _CACHE["last_result"] = res

    out = np.zeros((2, C), dtype=np.float64)
    for core in range(N_CORES):
        row = res.results[core]["out"].reshape(CPC, 2).astype(np.float64)
        for dr in range(2):
            out[dr, core * CPC:(core + 1) * CPC] = row[:, dr] / (NPC * s_y[dr])
    out[0] += xs0
    out[1] += xs1
    return out.astype(np.float32)


# revision 23
# speedup vs baseline: 1.3880x; 1.1456x over previous
"""Trainium2 Bass kernel for nn_ProtoCycleModel (retrieval_knn).

Problem: P=65536 prototypes, C=64 classes, D=256.
Per class c (rows c::64 of each table, n=1024):
    p2_inv = (p2_c - b) @ inv(W.T)          # y-side of direction "source"
    p1_fwd = p1_c @ W.T + b                 # y-side of direction "target"
    loss_src[c] = mean_i min_j ||p1_c[i] - p2_inv[j]||^2
    loss_tgt[c] = mean_i min_j ||p2_c[i] - p1_fwd[j]||^2
Output: (2, 64) fp32.

Sharding: class axis across 8 cores (8 classes/core).

v2 design (per core, per class, both directions dr in {0,1}):
  - Host passes class-major bf16 copies of both tables; the device loads
    them with XBAR DMA-transpose directly into d-major SBUF tiles
    xtb[t] = [128 d_lo, 2 d_chunk, 1024 i] bf16 (no PE transposes).
  - x~ = fp8(xtb)  (Act copy)  -> DoubleRow G stationary.
  - Transform (PE, fp32r stationary mats = -2*s_y[dr]*T_dr, moving bf16):
      pstf[dr,dcp] = -2*s_y*(linear part of y), bias added on the way out.
  - yt[dr] = fp8(pstf + bias)   [Act]  -> DoubleRow G moving.
  - sq = Square(pstf*sqrt_c + bias*sqrt_c) [Act], ysrow ones-matmul (PE,
    M=128 -> broadcast), ys_sb = copy to SBUF [DVE].  ys = s_y*|y_j|^2
    computed from the UNquantized transform (critical for accuracy).
  - G tile per (dr, i-tile): one DoubleRow fp8 matmul per 512-bank
    (K=256 in one instruction, 0.5 cycles/row).
  - min_j(G + ys): even i-tiles via DVE tensor_tensor_reduce (ys as the
    second operand), odd i-tiles via PE K=1 ys-fold matmul + gpsimd
    tensor_reduce(min) - balances DVE/Pool.
  - Scalar |x_i|^2 term and all unscaling are applied on the host
    (loss = psf/(1024*s_y) + mean_i|x_i|^2).
"""

import numpy as np

P, C, D = 65536, 64, 256
N_CORES = 8
CPC = C // N_CORES          # classes per core = 8
NPC = P // C                # prototypes per class = 1024
IT = NPC // 128             # i-tiles per class = 8

_CACHE = {}


def _build_bass():
    import concourse.bass as bass
    from concourse import bacc
    import concourse.tile as tile
    from concourse import mybir

    FP32 = mybir.dt.float32
    FP32R = mybir.dt.float32r
    BF16 = mybir.dt.bfloat16
    FP8 = mybir.dt.float8e4
    AF = mybir.ActivationFunctionType
    ALU = mybir.AluOpType
    AX = mybir.AxisListType
    DR_MODE = mybir.MatmulPerfMode.DoubleRow

    nc = bacc.Bacc(None, target_bir_lowering=False)

    p1b_d = nc.dram_tensor("p1b", [CPC * NPC, D], BF16, kind="ExternalInput")
    p2b_d = nc.dram_tensor("p2b", [CPC * NPC, D], BF16, kind="ExternalInput")
    # mats[dr][dc][dcp] : [128,128] bf16 lhsT chunk of -2*s_y[dr]*T_dr
    mats_d = nc.dram_tensor("mats", [2, 2, 2, 128, 128], BF16,
                            kind="ExternalInput")
    # constsr: [:,0:128] ones (ysrow lhsT / fold row), [:,128:129] ones col
    constsr_d = nc.dram_tensor("constsr", [128, 129], FP32R,
                               kind="ExternalInput")
    # constsf cols: 0-3 bias_dev[dr][dcp]; 4-7 bias_sq[dr][dcp]; 8-9 sqrt_c[dr]
    constsf_d = nc.dram_tensor("constsf", [128, 10], FP32, kind="ExternalInput")
    out_d = nc.dram_tensor("out", [1, 2 * CPC], FP32, kind="ExternalOutput")

    with tile.TileContext(nc) as tc:
        with (
            tc.tile_pool(name="const", bufs=1) as const,
            tc.tile_pool(name="xb", bufs=4) as xb_p,
            tc.tile_pool(name="xq", bufs=4) as xq_p,
            tc.tile_pool(name="yt", bufs=4) as yt_p,
            tc.tile_pool(name="sq", bufs=4) as sq_p,
            tc.tile_pool(name="ys", bufs=4) as ys_p,
            tc.tile_pool(name="gc", bufs=6) as gc_p,
            tc.tile_pool(name="psg", bufs=3, space="PSUM") as psg_p,
            tc.tile_pool(name="psm", bufs=1, space="PSUM") as psm_p,
        ):
            # const tiles; their DMAs are issued after the first class
            # loads so the table transposes get the early DMA sem slots.
            cr = const.tile([128, 129], FP32R)
            cf = const.tile([128, 10], FP32)
            mats = const.tile([128, 2, 2, 2, 128], BF16)

            def emit_const_dmas():
                nc.scalar.dma_start(
                    mats[:], mats_d[:].rearrange("a b c p d -> p a b c d"))
                nc.scalar.dma_start(cf[:], constsf_d[:])
                nc.scalar.dma_start(cr[:], constsr_d[:])
            ones128 = cr[:, 0:128]
            ones_row = cr[0:1, 0:128]
            ones_col = cr[:, 128:129]

            # per-unit min columns: col = (ci*2 + dr)*8 + it.  All accums
            # come from DVE tensor_scalar ops (in-order engine).
            pmin = const.tile([128, 128], FP32, name="pmin")
            dumf = const.tile([128, 1], FP32, name="dumf")
            dumb = const.tile([128, 1], BF16, name="dumb")

            state = {}

            def emit_dmas(c):
                """DMA-transpose class-c rows of both tables into d-major
                (p2 first: dir0's transforms consume it first)."""
                halves = {}
                for t, src in ((1, p2b_d), (0, p1b_d)):
                    xtb = xb_p.tile([128, 2, NPC], BF16, tag=f"xb{t}",
                                    name=f"xtb{t}")
                    nc.sync.dma_start_transpose(
                        xtb[:], src[c * NPC:(c + 1) * NPC, :])
                    halves[t] = xtb
                state[("xtb", c)] = [halves[0], halves[1]]

            def prep_ops(c):
                """Generator of (engine_tag, thunk) prep ops for class c."""
                xtbs = state[("xtb", c)]
                xqs = [None, None]
                yts = [None, None]
                yss = [None, None]
                sqs = [[None, None], [None, None]]
                psys = [None, None]
                pstfs = [[None, None], [None, None]]
                state[("res", c)] = (xqs, yts, yss)

                def quant(t):
                    xq = xq_p.tile([128, 2, NPC], FP8, tag=f"xq{t}",
                                   name=f"xq{t}")
                    nc.gpsimd.dma_start(xq[:], xtbs[t][:])
                    xqs[t] = xq

                def tf(dr, dcp, dc, jh):
                    if dc == 0 and jh == 0:
                        pstfs[dr][dcp] = psm_p.tile(
                            [128, NPC], FP32, tag="psm", name="pstf")
                    src = xtbs[1 - dr]  # dir0 transforms p2, dir1 p1
                    nc.tensor.matmul(
                        pstfs[dr][dcp][:, jh * 512:(jh + 1) * 512],
                        mats[:, dr, dc, dcp, :],
                        src[:, dc, jh * 512:(jh + 1) * 512],
                        start=(dc == 0), stop=(dc == 1),
                    )

                def ytf(dr, dcp):
                    if dcp == 0:
                        yts[dr] = yt_p.tile([128, 2, NPC], FP8, tag=f"yt{dr}", name=f"yt{dr}")
                    nc.scalar.activation(
                        yts[dr][:, dcp, :], pstfs[dr][dcp][:], AF.Identity,
                        bias=cf[:, dr * 2 + dcp:dr * 2 + dcp + 1], scale=1.0)

                def sqf(dr, dcp):
                    sq_t = sq_p.tile([128, NPC], FP32R, tag="sq", name="sq_t")
                    nc.scalar.activation(
                        sq_t[:], pstfs[dr][dcp][:], AF.Square,
                        bias=cf[:, 4 + dr * 2 + dcp:4 + dr * 2 + dcp + 1],
                        scale=cf[:, 8 + dr:9 + dr])
                    sqs[dr][dcp] = sq_t

                def ysrow(dr, jh, dcp):
                    if jh == 0 and dcp == 0:
                        psys[dr] = psm_p.tile([128, NPC], FP32, tag="psm", name="psy")
                    nc.tensor.matmul(
                        psys[dr][:, jh * 512:(jh + 1) * 512],
                        ones128,
                        sqs[dr][dcp][:, jh * 512:(jh + 1) * 512],
                        start=(dcp == 0), stop=(dcp == 1),
                    )

                def ysb(dr):
                    ys_t = ys_p.tile([128, NPC], FP32R, tag=f"ys{dr}", name=f"ys{dr}")
                    nc.scalar.copy(ys_t[:], psys[dr][:])
                    yss[dr] = ys_t

                # (engine, thunk) in dependency order; engine tags are only
                # informational for interleaving.
                ops = []
                ops.append(("act", lambda: quant(0)))
                ops.append(("act", lambda: quant(1)))
                for dr in range(2):
                    for dcp in range(2):
                        for jh in range(2):
                            for dc in range(2):
                                ops.append(("pe", (lambda a, b_, c_, d_:
                                            lambda: tf(a, b_, c_, d_))(
                                                dr, dcp, dc, jh)))
                        ops.append(("act", (lambda a, b_: lambda: ytf(a, b_))(
                            dr, dcp)))
                        ops.append(("act", (lambda a, b_: lambda: sqf(a, b_))(
                            dr, dcp)))
                    for jh in range(2):
                        for dcp in range(2):
                            ops.append(("pe", (lambda a, b_, c_:
                                        lambda: ysrow(a, b_, c_))(
                                            dr, jh, dcp)))
                    ops.append(("dve", (lambda a: lambda: ysb(a))(dr)))
                return ops

            def pairwise_units(c):
                """Generator of per-unit thunks for class c."""
                xqs, yts, yss = state.pop(("res", c))
                state.pop(("xtb", c))
                ci = c

                def unit(dr, it):
                    xside = xqs[dr]     # dir0 x = p1, dir1 x = p2
                    pg = psg_p.tile([128, NPC], FP32, tag="g", name="pg")
                    for jh in range(2):
                        # DoubleRow fp8 G (K=256 in one op), then fold ys
                        # into the same psum bank via a K=1 ones matmul.
                        nc.tensor.matmul(
                            pg[:, jh * 512:(jh + 1) * 512],
                            xside[:, :, it * 128:(it + 1) * 128],
                            yts[dr][:, :, jh * 512:(jh + 1) * 512],
                            start=True, stop=False,
                            perf_mode=DR_MODE,
                        )
                        nc.tensor.matmul(
                            pg[:, jh * 512:(jh + 1) * 512],
                            ones_row,
                            yss[dr][0:1, jh * 512:(jh + 1) * 512],
                            start=False, stop=True,
                        )
                    col = (ci * 2 + dr) * 8 + it
                    u = dr * 8 + it
                    if u % 3 != 2:
                        # direct: DVE min-reduce straight from PSUM
                        nc.vector.tensor_scalar(
                            out=dumf.broadcast_to((128, NPC)),
                            in0=pg[:], scalar1=0.0, scalar2=None,
                            op0=ALU.add, op1=ALU.min,
                            accum_out=pmin[:, col:col + 1])
                        return None
                    # offloaded: Act copies psum->bf16 SBUF; the DVE 4x min
                    # is deferred to the class end (keeps the DVE stream hot)
                    gc = gc_p.tile([128, NPC], BF16, tag="gc", name="gc")
                    nc.scalar.copy(gc[:], pg[:])

                    def dmin():
                        nc.vector.tensor_scalar(
                            out=dumb.broadcast_to((128, NPC)),
                            in0=gc[:], scalar1=0.0, scalar2=None,
                            op0=ALU.add, op1=ALU.min,
                            accum_out=pmin[:, col:col + 1])
                    return dmin

                return [(dr, it, (lambda a, b_: lambda: unit(a, b_))(dr, it))
                        for dr in (0, 1) for it in range(8)]

            # ---- software-pipelined main loop ----
            # D-path units write pminD cols with running-min per slot?  No:
            # each (dr, it) has its own (col, path) slot: it//2 in 0..3,
            # even it -> D, odd it -> P.  Each col written exactly twice?
            # it=0,2 -> slots 0,1 (D); it=4,6 -> slots 2,3 (D);
            # it=1,3 -> slots 0,1 (P); it=5,7 -> slots 2,3 (P).  Unique. OK.
            emit_dmas(0)
            emit_dmas(1)
            emit_const_dmas()
            prep_queue = prep_ops(0)
            for op in prep_queue:
                op[1]()
            for c in range(CPC):
                units = pairwise_units(c)
                if c + 2 < CPC:
                    emit_dmas(c + 2)
                if c + 1 < CPC:
                    prep_queue = prep_ops(c + 1)
                else:
                    prep_queue = []
                # interleave: after each unit, emit a slice of prep ops
                # (front-loaded into the first 12 unit slots); deferred
                # alpha-unit mins run at the end of the class.
                nu = len(units)
                np_ops = len(prep_queue)
                done = 0
                deferred = []
                for ui, (dr, it, thunk) in enumerate(units):
                    d = thunk()
                    if d is not None:
                        deferred.append(d)
                    want = min(np_ops, (ui + 1) * np_ops // 12)
                    while done < want:
                        prep_queue[done][1]()
                        done += 1
                for d in deferred:
                    d()

            # ---- finals ----
            # sum the 8 i-tiles per (class, dir), then cross-partition sum.
            red = const.tile([128, 16], FP32R, name="red")
            with nc.allow_low_precision(reason="fp32r is bit-identical fp32"):
                nc.vector.tensor_reduce(
                    out=red[:],
                    in_=pmin[:].rearrange("p (g k) -> p g k", k=8),
                    axis=AX.X, op=ALU.add)
            psf = psm_p.tile([1, 16], FP32, tag="psm", name="psf")
            nc.tensor.matmul(psf[:], ones_col, red[:], start=True, stop=True)
            outrow = const.tile([1, 16], FP32)
            nc.scalar.copy(outrow[:], psf[:])
            nc.sync.dma_start(out_d[:], outrow[:])

    nc.compile()
    return nc


def _get_nc():
    if "nc" not in _CACHE:
        _CACHE["nc"] = _build_bass()
    return _CACHE["nc"]


def kernel(protos1, protos2, W, b, num_classes):
    import ml_dtypes
    from concourse.bass_utils import run_bass_kernel_spmd

    nc_classes = int(num_classes)
    assert nc_classes == C and protos1.shape == (P, D)

    protos1 = np.ascontiguousarray(protos1, dtype=np.float32)
    protos2 = np.ascontiguousarray(protos2, dtype=np.float32)
    W = np.asarray(W, dtype=np.float32)
    b = np.asarray(b, dtype=np.float32)

    # host-side prep: inverse, scales, transform matrices
    V = np.linalg.inv(W.T.astype(np.float64)).astype(np.float32)  # (p2-b)@V
    B0 = (np.linalg.norm(protos2 - b, axis=1).max()
          * np.linalg.norm(V, axis=0).max())
    B1 = (np.linalg.norm(protos1, axis=1).max()
          * np.linalg.norm(W, axis=1).max() + np.abs(b).max())
    s_y = np.array([56.0 / B0, 56.0 / B1], np.float64)

    import ml_dtypes as _mld
    mats = np.zeros((2, 2, 2, 128, 128), _mld.bfloat16)
    for dr, T in ((0, V), (1, W.T.copy())):
        M = (-2.0 * s_y[dr]) * T.astype(np.float64)
        for dc in range(2):
            for dcp in range(2):
                mats[dr, dc, dcp] = M[dc * 128:(dc + 1) * 128,
                                      dcp * 128:(dcp + 1) * 128]

    bias_dev = np.zeros((2, 256), np.float64)
    bias_dev[0] = 2.0 * s_y[0] * (b.astype(np.float64) @ V.astype(np.float64))
    bias_dev[1] = -2.0 * s_y[1] * b
    sqrt_c = np.sqrt(1.0 / (4.0 * s_y))          # per dir

    constsf = np.zeros((128, 10), np.float32)
    for dr in range(2):
        for dcp in range(2):
            col = bias_dev[dr, dcp * 128:(dcp + 1) * 128]
            constsf[:, dr * 2 + dcp] = col
            constsf[:, 4 + dr * 2 + dcp] = col * sqrt_c[dr]
        constsf[:, 8 + dr] = sqrt_c[dr]
    constsr = np.concatenate(
        [np.ones((128, 128), np.float32), np.ones((128, 1), np.float32)],
        axis=1)

    # class-major reordering: (P, D) -> (C, NPC, D), bf16 copies
    p1c = np.ascontiguousarray(protos1.reshape(NPC, C, D).transpose(1, 0, 2))
    p2c = np.ascontiguousarray(protos2.reshape(NPC, C, D).transpose(1, 0, 2))
    p1bf = p1c.astype(ml_dtypes.bfloat16)
    p2bf = p2c.astype(ml_dtypes.bfloat16)

    # host-side |x|^2 means per (dir, class)
    xs0 = (p1c.astype(np.float64) ** 2).sum(axis=2).mean(axis=1)  # (C,)
    xs1 = (p2c.astype(np.float64) ** 2).sum(axis=2).mean(axis=1)

    in_maps = []
    for core in range(N_CORES):
        sl = slice(core * CPC, (core + 1) * CPC)
        in_maps.append({
            "p1b": np.ascontiguousarray(p1bf[sl].reshape(CPC * NPC, D)),
            "p2b": np.ascontiguousarray(p2bf[sl].reshape(CPC * NPC, D)),
            "mats": mats,
            "constsr": constsr,
            "constsf": constsf,
        })

    nc = _get_nc()
    res = run_bass_kernel_spmd(nc, in_maps, core_ids=list(range(N_CORES)))
    _CACHE["last_result"] = res

    out = np.zeros((2, C), dtype=np.float64)
    for core in range(N_CORES):
        row = res.results[core]["out"].reshape(CPC, 2).astype(np.float64)
        for dr in range(2):
            out[dr, core * CPC:(core + 1) * CPC] = row[:, dr] / (NPC * s_y[dr])
    out[0] += xs0
    out[1] += xs1
    return out.astype(np.float32)


# revision 26
# speedup vs baseline: 1.4079x; 1.0143x over previous
"""Trainium2 Bass kernel for nn_ProtoCycleModel (retrieval_knn).

Problem: P=65536 prototypes, C=64 classes, D=256.
Per class c (rows c::64 of each table, n=1024):
    p2_inv = (p2_c - b) @ inv(W.T)          # y-side of direction "source"
    p1_fwd = p1_c @ W.T + b                 # y-side of direction "target"
    loss_src[c] = mean_i min_j ||p1_c[i] - p2_inv[j]||^2
    loss_tgt[c] = mean_i min_j ||p2_c[i] - p1_fwd[j]||^2
Output: (2, 64) fp32.

Sharding: class axis across 8 cores (8 classes/core).

Design (per core, per class, both directions dr in {0,1}):
  - Host passes class-major bf16 copies of both tables; the device loads
    them with XBAR DMA-transpose directly into d-major SBUF tiles
    xtb[t] = [128 d_lo, 2 d_chunk, 1024 i] bf16 (no PE transposes).
  - x~ = fp8(xtb) via gpsimd cast-DMA -> DoubleRow G stationary.
  - Transform (PE, bf16 stationary mats = -2*s_y[dr]*T_dr, bf16 moving):
      pstf[dr,dcp] = -2*s_y*(linear part of y).
  - yt[dr] = fp8(pstf + bias)  [Act]  -> DoubleRow G moving.
  - sq = Square(pstf*sqrt_c + bias*sqrt_c) [Act], ysrow ones-matmul (PE,
    M=128), ys_sb = copy to SBUF [Act].  ys = s_y*|y_j|^2 is computed
    from the UNquantized transform (critical for dir0 accuracy; fp8
    noise through inv(W.T) would otherwise dominate).
  - G tile per (dr, i-tile): one DoubleRow fp8 matmul per 512-bank
    (K=256 in one instruction, 0.5 cycles/row), then a K=1 ones-matmul
    folds ys into the same psum bank.
  - min_j(G + ys): DVE tensor_scalar(op1=min, accum_out) straight from
    PSUM for 11/16 i-tiles; for the other 5, Act copies psum->bf16 SBUF
    and the DVE min runs in 4x_2p mode (balances DVE vs Act).
    (tensor_tensor_reduce and all gpsimd ALU ops are rejected by this
    toolchain's codegen/backend, so DVE is the only reducer.)
  - Software-pipelined: prep of class c+1 interleaves with the pairwise
    mins of class c; DMA-transposes prefetch two classes ahead.
  - Scalar |x_i|^2 term and all unscaling are applied on the host
    (loss = psf/(1024*s_y) + mean_i|x_i|^2).
"""

import numpy as np

P, C, D = 65536, 64, 256
N_CORES = 8
CPC = C // N_CORES          # classes per core = 8
NPC = P // C                # prototypes per class = 1024
IT = NPC // 128             # i-tiles per class = 8

_CACHE = {}


def _build_bass():
    import concourse.bass as bass
    from concourse import bacc
    import concourse.tile as tile
    from concourse import mybir

    FP32 = mybir.dt.float32
    FP32R = mybir.dt.float32r
    BF16 = mybir.dt.bfloat16
    FP8 = mybir.dt.float8e4
    AF = mybir.ActivationFunctionType
    ALU = mybir.AluOpType
    AX = mybir.AxisListType
    DR_MODE = mybir.MatmulPerfMode.DoubleRow

    nc = bacc.Bacc(None, target_bir_lowering=False)

    p1b_d = nc.dram_tensor("p1b", [CPC * NPC, D], BF16, kind="ExternalInput")
    p2b_d = nc.dram_tensor("p2b", [CPC * NPC, D], BF16, kind="ExternalInput")
    # mats[dr][dc][dcp] : [128,128] bf16 lhsT chunk of -2*s_y[dr]*T_dr
    mats_d = nc.dram_tensor("mats", [2, 2, 2, 128, 128], BF16,
                            kind="ExternalInput")
    # constsr: [:,0:128] ones (ysrow lhsT / fold row), [:,128:129] ones col
    constsr_d = nc.dram_tensor("constsr", [128, 129], FP32R,
                               kind="ExternalInput")
    # constsf cols: 0-3 bias_dev[dr][dcp]; 4-7 bias_sq[dr][dcp]; 8-9 sqrt_c[dr]
    constsf_d = nc.dram_tensor("constsf", [128, 10], FP32, kind="ExternalInput")
    out_d = nc.dram_tensor("out", [1, 2 * CPC], FP32, kind="ExternalOutput")

    with tile.TileContext(nc) as tc:
        with (
            tc.tile_pool(name="const", bufs=1) as const,
            tc.tile_pool(name="xb", bufs=4) as xb_p,
            tc.tile_pool(name="xq", bufs=4) as xq_p,
            tc.tile_pool(name="yt", bufs=4) as yt_p,
            tc.tile_pool(name="sq", bufs=4) as sq_p,
            tc.tile_pool(name="ys", bufs=4) as ys_p,
            tc.tile_pool(name="gc", bufs=6) as gc_p,
            tc.tile_pool(name="psg", bufs=3, space="PSUM") as psg_p,
            tc.tile_pool(name="psm", bufs=1, space="PSUM") as psm_p,
        ):
            # const tiles; their DMAs are issued after the first class
            # loads so the table transposes get the early DMA sem slots.
            cr = const.tile([128, 129], FP32R)
            cf = const.tile([128, 10], FP32)
            mats = const.tile([128, 2, 2, 2, 128], BF16)

            def emit_const_dmas():
                nc.scalar.dma_start(
                    mats[:], mats_d[:].rearrange("a b c p d -> p a b c d"))
                nc.scalar.dma_start(cf[:], constsf_d[:])
                nc.scalar.dma_start(cr[:], constsr_d[:])
            ones128 = cr[:, 0:128]
            ones_row = cr[0:1, 0:128]
            ones_col = cr[:, 128:129]

            # per-unit min columns: col = (ci*2 + dr)*8 + it.  All accums
            # come from DVE tensor_scalar ops (in-order engine).
            pmin = const.tile([128, 128], FP32, name="pmin")
            dumf = const.tile([128, 1], FP32, name="dumf")
            dumb = const.tile([128, 1], BF16, name="dumb")

            state = {}

            def emit_dmas(c):
                """DMA-transpose class-c rows of both tables into d-major
                (p2 first: dir0's transforms consume it first)."""
                halves = {}
                for t, src in ((1, p2b_d), (0, p1b_d)):
                    xtb = xb_p.tile([128, 2, NPC], BF16, tag=f"xb{t}",
                                    name=f"xtb{t}")
                    nc.sync.dma_start_transpose(
                        xtb[:], src[c * NPC:(c + 1) * NPC, :])
                    halves[t] = xtb
                state[("xtb", c)] = [halves[0], halves[1]]

            def prep_ops(c):
                """Generator of (engine_tag, thunk) prep ops for class c."""
                pp = psg_p if c == 0 else psm_p
                ptag = "g" if c == 0 else "psm"
                xtbs = state[("xtb", c)]
                xqs = [None, None]
                yts = [None, None]
                yss = [None, None]
                sqs = [[None, None], [None, None]]
                psys = [None, None]
                pstfs = [[None, None], [None, None]]
                state[("res", c)] = (xqs, yts, yss)

                def quant(t):
                    xq = xq_p.tile([128, 2, NPC], FP8, tag=f"xq{t}",
                                   name=f"xq{t}")
                    nc.gpsimd.dma_start(xq[:], xtbs[t][:])
                    xqs[t] = xq

                def tf(dr, dcp, dc, jh):
                    if dc == 0 and jh == 0:
                        pstfs[dr][dcp] = pp.tile(
                            [128, NPC], FP32, tag=ptag, name="pstf")
                    src = xtbs[1 - dr]  # dir0 transforms p2, dir1 p1
                    nc.tensor.matmul(
                        pstfs[dr][dcp][:, jh * 512:(jh + 1) * 512],
                        mats[:, dr, dc, dcp, :],
                        src[:, dc, jh * 512:(jh + 1) * 512],
                        start=(dc == 0), stop=(dc == 1),
                    )

                def ytf(dr, dcp):
                    if dcp == 0:
                        yts[dr] = yt_p.tile([128, 2, NPC], FP8, tag=f"yt{dr}", name=f"yt{dr}")
                    nc.scalar.activation(
                        yts[dr][:, dcp, :], pstfs[dr][dcp][:], AF.Identity,
                        bias=cf[:, dr * 2 + dcp:dr * 2 + dcp + 1], scale=1.0)

                def sqf(dr, dcp):
                    sq_t = sq_p.tile([128, NPC], FP32R, tag="sq", name="sq_t")
                    nc.scalar.activation(
                        sq_t[:], pstfs[dr][dcp][:], AF.Square,
                        bias=cf[:, 4 + dr * 2 + dcp:4 + dr * 2 + dcp + 1],
                        scale=cf[:, 8 + dr:9 + dr])
                    sqs[dr][dcp] = sq_t

                def ysrow(dr, jh, dcp):
                    if jh == 0 and dcp == 0:
                        psys[dr] = pp.tile([128, NPC], FP32, tag=ptag, name="psy")
                    nc.tensor.matmul(
                        psys[dr][:, jh * 512:(jh + 1) * 512],
                        ones128,
                        sqs[dr][dcp][:, jh * 512:(jh + 1) * 512],
                        start=(dcp == 0), stop=(dcp == 1),
                    )

                def ysb(dr):
                    ys_t = ys_p.tile([128, NPC], FP32R, tag=f"ys{dr}", name=f"ys{dr}")
                    nc.scalar.copy(ys_t[:], psys[dr][:])
                    yss[dr] = ys_t

                # (engine, thunk) in dependency order; engine tags are only
                # informational for interleaving.
                ops = []
                ops.append(("act", lambda: quant(0)))
                ops.append(("act", lambda: quant(1)))
                for dr in range(2):
                    for dcp in range(2):
                        for jh in range(2):
                            for dc in range(2):
                                ops.append(("pe", (lambda a, b_, c_, d_:
                                            lambda: tf(a, b_, c_, d_))(
                                                dr, dcp, dc, jh)))
                        ops.append(("act", (lambda a, b_: lambda: ytf(a, b_))(
                            dr, dcp)))
                        ops.append(("act", (lambda a, b_: lambda: sqf(a, b_))(
                            dr, dcp)))
                    for jh in range(2):
                        for dcp in range(2):
                            ops.append(("pe", (lambda a, b_, c_:
                                        lambda: ysrow(a, b_, c_))(
                                            dr, jh, dcp)))
                    ops.append(("dve", (lambda a: lambda: ysb(a))(dr)))
                return ops

            def pairwise_units(c):
                """Generator of per-unit thunks for class c."""
                xqs, yts, yss = state.pop(("res", c))
                state.pop(("xtb", c))
                ci = c

                def unit(dr, it):
                    xside = xqs[dr]     # dir0 x = p1, dir1 x = p2
                    pg = psg_p.tile([128, NPC], FP32, tag="g", name="pg")
                    for jh in range(2):
                        # DoubleRow fp8 G (K=256 in one op), then fold ys
                        # into the same psum bank via a K=1 ones matmul.
                        nc.tensor.matmul(
                            pg[:, jh * 512:(jh + 1) * 512],
                            xside[:, :, it * 128:(it + 1) * 128],
                            yts[dr][:, :, jh * 512:(jh + 1) * 512],
                            start=True, stop=False,
                            perf_mode=DR_MODE,
                        )
                        nc.tensor.matmul(
                            pg[:, jh * 512:(jh + 1) * 512],
                            ones_row,
                            yss[dr][0:1, jh * 512:(jh + 1) * 512],
                            start=False, stop=True,
                        )
                    col = (ci * 2 + dr) * 8 + it
                    u = dr * 8 + it
                    if u % 3 != 2:
                        # direct: DVE min-reduce straight from PSUM
                        nc.vector.tensor_scalar(
                            out=dumf.broadcast_to((128, NPC)),
                            in0=pg[:], scalar1=0.0, scalar2=None,
                            op0=ALU.add, op1=ALU.min,
                            accum_out=pmin[:, col:col + 1])
                        return None
                    # offloaded: Act copies psum->bf16 SBUF; the DVE 4x min
                    # is deferred to the class end (keeps the DVE stream hot)
                    gc = gc_p.tile([128, NPC], BF16, tag="gc", name="gc")
                    nc.scalar.copy(gc[:], pg[:])
                    nc.vector.tensor_scalar(
                        out=dumb.broadcast_to((128, NPC)),
                        in0=gc[:], scalar1=0.0, scalar2=None,
                        op0=ALU.add, op1=ALU.min,
                        accum_out=pmin[:, col:col + 1])
                    return None

                return [(dr, it, (lambda a, b_: lambda: unit(a, b_))(dr, it))
                        for dr in (0, 1) for it in range(8)]

            # ---- software-pipelined main loop ----
            # D-path units write pminD cols with running-min per slot?  No:
            # each (dr, it) has its own (col, path) slot: it//2 in 0..3,
            # even it -> D, odd it -> P.  Each col written exactly twice?
            # it=0,2 -> slots 0,1 (D); it=4,6 -> slots 2,3 (D);
            # it=1,3 -> slots 0,1 (P); it=5,7 -> slots 2,3 (P).  Unique. OK.
            emit_dmas(0)
            emit_dmas(1)
            emit_const_dmas()
            prep_queue = prep_ops(0)
            for op in prep_queue:
                op[1]()
            for c in range(CPC):
                units = pairwise_units(c)
                if c + 2 < CPC:
                    emit_dmas(c + 2)
                if c + 1 < CPC:
                    prep_queue = prep_ops(c + 1)
                else:
                    prep_queue = []
                # interleave: after each unit, emit a slice of prep ops
                # (front-loaded into the first 12 unit slots); deferred
                # alpha-unit mins run at the end of the class.
                nu = len(units)
                np_ops = len(prep_queue)
                done = 0
                deferred = []
                for ui, (dr, it, thunk) in enumerate(units):
                    d = thunk()
                    if d is not None:
                        deferred.append(d)
                    want = min(np_ops, (ui + 1) * np_ops // 12)
                    while done < want:
                        prep_queue[done][1]()
                        done += 1
                for d in deferred:
                    d()

            # ---- finals ----
            # sum the 8 i-tiles per (class, dir), then cross-partition sum.
            red = const.tile([128, 16], FP32R, name="red")
            with nc.allow_low_precision(reason="fp32r is bit-identical fp32"):
                nc.vector.tensor_reduce(
                    out=red[:],
                    in_=pmin[:].rearrange("p (g k) -> p g k", k=8),
                    axis=AX.X, op=ALU.add)
            psf = psm_p.tile([1, 16], FP32, tag="psm", name="psf")
            nc.tensor.matmul(psf[:], ones_col, red[:], start=True, stop=True)
            outrow = const.tile([1, 16], FP32)
            nc.scalar.copy(outrow[:], psf[:])
            nc.sync.dma_start(out_d[:], outrow[:])

    nc.compile()
    return nc


def _get_nc():
    if "nc" not in _CACHE:
        _CACHE["nc"] = _build_bass()
    return _CACHE["nc"]


def kernel(protos1, protos2, W, b, num_classes):
    import ml_dtypes
    from concourse.bass_utils import run_bass_kernel_spmd

    nc_classes = int(num_classes)
    assert nc_classes == C and protos1.shape == (P, D)

    protos1 = np.ascontiguousarray(protos1, dtype=np.float32)
    protos2 = np.ascontiguousarray(protos2, dtype=np.float32)
    W = np.asarray(W, dtype=np.float32)
    b = np.asarray(b, dtype=np.float32)

    # host-side prep: inverse, scales, transform matrices
    V = np.linalg.inv(W.T.astype(np.float64)).astype(np.float32)  # (p2-b)@V
    B0 = (np.linalg.norm(protos2 - b, axis=1).max()
          * np.linalg.norm(V, axis=0).max())
    B1 = (np.linalg.norm(protos1, axis=1).max()
          * np.linalg.norm(W, axis=1).max() + np.abs(b).max())
    s_y = np.array([56.0 / B0, 56.0 / B1], np.float64)

    import ml_dtypes as _mld
    mats = np.zeros((2, 2, 2, 128, 128), _mld.bfloat16)
    for dr, T in ((0, V), (1, W.T.copy())):
        M = (-2.0 * s_y[dr]) * T.astype(np.float64)
        for dc in range(2):
            for dcp in range(2):
                mats[dr, dc, dcp] = M[dc * 128:(dc + 1) * 128,
                                      dcp * 128:(dcp + 1) * 128]

    bias_dev = np.zeros((2, 256), np.float64)
    bias_dev[0] = 2.0 * s_y[0] * (b.astype(np.float64) @ V.astype(np.float64))
    bias_dev[1] = -2.0 * s_y[1] * b
    sqrt_c = np.sqrt(1.0 / (4.0 * s_y))          # per dir

    constsf = np.zeros((128, 10), np.float32)
    for dr in range(2):
        for dcp in range(2):
            col = bias_dev[dr, dcp * 128:(dcp + 1) * 128]
            constsf[:, dr * 2 + dcp] = col
            constsf[:, 4 + dr * 2 + dcp] = col * sqrt_c[dr]
        constsf[:, 8 + dr] = sqrt_c[dr]
    constsr = np.concatenate(
        [np.ones((128, 128), np.float32), np.ones((128, 1), np.float32)],
        axis=1)

    # class-major reordering: (P, D) -> (C, NPC, D), bf16 copies
    p1c = np.ascontiguousarray(protos1.reshape(NPC, C, D).transpose(1, 0, 2))
    p2c = np.ascontiguousarray(protos2.reshape(NPC, C, D).transpose(1, 0, 2))
    p1bf = p1c.astype(ml_dtypes.bfloat16)
    p2bf = p2c.astype(ml_dtypes.bfloat16)

    # host-side |x|^2 means per (dir, class)
    xs0 = (p1c.astype(np.float64) ** 2).sum(axis=2).mean(axis=1)  # (C,)
    xs1 = (p2c.astype(np.float64) ** 2).sum(axis=2).mean(axis=1)

    in_maps = []
    for core in range(N_CORES):
        sl = slice(core * CPC, (core + 1) * CPC)
        in_maps.append({
            "p1b": np.ascontiguousarray(p1bf[sl].reshape(CPC * NPC, D)),
            "p2b": np.ascontiguousarray(p2bf[sl].reshape(CPC * NPC, D)),
            "mats": mats,
            "constsr": constsr,
            "constsf": constsf,
        })

    nc = _get_nc()
    res = run_bass_kernel_spmd(nc, in_maps, core_ids=list(range(N_CORES)))
    _CACHE["last_result"] = res

    out = np.zeros((2, C), dtype=np.float64)
    for core in range(N_CORES):
        row = res.results[core]["out"].reshape(CPC, 2).astype(np.float64)
        for dr in range(2):
            out[dr, core * CPC:(core + 1) * CPC] = row[:, dr] / (NPC * s_y[dr])
    out[0] += xs0
    out[1] += xs1
    return out.astype(np.float32)
